# revision 43
# baseline (speedup 1.0000x reference)
"""Trainium2 Bass kernel for the additive-attention module.

Per-core computation (data-parallel over batch, 8 cores, 32 batches each):
  att_enc = enc @ W_enc            [6272, 2048] @ [2048, 512]  (dominant matmul)
  att_dec = dec @ W_dec + b_dec    [32, 512] @ [512, 512]
  hidden  = relu(att_enc + att_dec[b] + b_enc)
  att     = hidden @ W_fin         (b_fin dropped: softmax shift-invariant)
  w       = softmax_p(att)         (no max-subtraction: |att| < ~2)
  out     = sum_p w[b,p] * enc[b,p,:]

Dataflow per 128-row chunk c (4-chunk groups; nat DMA 3 groups ahead, fp8
cast + transpose 2 groups ahead; 26 groups total vs 26+ at GW=2 — bigger
groups amortize the per-group DMA->cast->transpose latency chain, which the
tile scheduler otherwise lock-steps against the PE):
  nat_c   [128 rows, 2048 e] bf16   gpsimd cast-DMA from f32 HBM (one DMA
                                    per group: finer splits cost more in
                                    desc-gen + sem overhead than they save)
  nat8_c  [128 rows, 2048 e] fp8e4  DVE-only casts of nat (the Act queue
                                    carries the PE-dependent relu/exp/w6
                                    chain; in-order Act would drag these
                                    prefetch casts behind it and lock-step
                                    the DMA pipeline to the PE)
  enc8T   [128 e-pairs, ...] fp8e4  half-width uint16 pair DMA-transpose of
                                    nat8: block k holds e-pairs (256k+2p,
                                    256k+2p+1) as adjacent bytes, matching
                                    the (even,odd)-row-paired W_enc load
  step1: att_encT accumulated per aj over bias-indicator matmul FIRST
         (bias16.T @ ind is ready long before enc8T, so the PE opens the
         accumulation without waiting on the transpose) + 8 fp8 DoubleRow
         k-pair matmuls (W_enc pre-scaled x16 into fp8, staged through one
         half-sized bf16 buffer; segment-free relus on Act) -> hidT bf16
  step4: att columns for group g-1, issued at the TOP of group g (hidT(g-1)
         is long ready, so these tiny matmuls fill the front of any enc8T
         stall and keep the PE p-state warm): ONE PSUM accumulation group
         (rank-1 zeroing matmul covers all columns first — a start=False
         first write would read uninitialized PSUM on HW; at_ps rides the
         mm_ps ring — packing it into the z bank corrupts the kernel-long z
         accumulation on HW); exp with scale=1/16 -> attT f32
  step6: w6 = mask_c * exp column, built on the SAME engine as the exp
         (Act, zero cross-engine hop) from prologue-precomputed masks
         (built AFTER the prefetch desc-gens — 10us of Pool selects must
         not block the early loads); chunk c releases at group h_c + 2; all
         chunks accumulate single-strip into out_ps[eg][0:32] / z_ps, and
         the tail is just reciprocal + alternating Act/DVE scaled copies,
         each overlapped with its own out-DMA slice.
"""

import sys

try:
    import concourse.bass as bass  # noqa: F401
except ImportError:
    sys.path.insert(0, "/opt/trn_rl_repo")

import numpy as np

import concourse.bass as bass
import concourse.mybir as mybir
import concourse.tile as tile
from concourse import bacc
from concourse.bass_utils import run_bass_kernel_spmd
from concourse.masks import make_identity

F32 = mybir.dt.float32
BF16 = mybir.dt.bfloat16
FP8 = mybir.dt.float8e4
AF = mybir.ActivationFunctionType
DR = mybir.MatmulPerfMode.DoubleRow

N_CORES = 8
B = 256
B_LOC = B // N_CORES  # 32
P = 196
E = 2048
A = 512
W = 512
ROWS = B_LOC * P  # 6272
NCHUNK = (ROWS + 127) // 128  # 49
EJ = E // 128  # 16
AJ = A // 128  # 4
WJ = W // 128  # 4
EG = E // 512  # 4
WSCALE = 16.0  # fp8 W_enc pre-scale (undone in the relu activation)
GW = 4  # chunks per steady-state group
PREFETCH = 3  # groups of nat DMA issued ahead of compute (nat ring holds 5:
# window = PREFETCH + 2 groups of liveness (step6 runs at h+2) + 1 slack)


def _batch_segments(r0, nrows):
    """Batch segments of global row range [r0, r0+nrows): (batch, local_s0, local_s1)."""
    segs = []
    b0 = r0 // P
    b1 = (r0 + nrows - 1) // P
    for b in range(b0, b1 + 1):
        s0 = max(b * P - r0, 0)
        s1 = min((b + 1) * P - r0, nrows)
        if s1 > s0:
            segs.append((b, s0, s1))
    return segs


def build():
    nc = bacc.Bacc()

    enc_x = nc.dram_tensor("encoder_out", [ROWS, E], F32, kind="ExternalInput")
    dec_x = nc.dram_tensor("decoder_out", [B_LOC, W], F32, kind="ExternalInput")
    wenc_x = nc.dram_tensor("W_enc", [E, A], F32, kind="ExternalInput")
    benc_x = nc.dram_tensor("b_enc", [1, A], F32, kind="ExternalInput")
    wdec_x = nc.dram_tensor("W_dec", [W, A], F32, kind="ExternalInput")
    bdec_x = nc.dram_tensor("b_dec", [1, A], F32, kind="ExternalInput")
    wfin_x = nc.dram_tensor("W_fin", [A], F32, kind="ExternalInput")
    out_x = nc.dram_tensor("out", [B_LOC, E], F32, kind="ExternalOutput")

    GR = GW * 128  # steady-state group row width

    with tile.TileContext(nc) as tc:
        with tc.tile_pool(name="consts", bufs=1) as consts:
            # tiles declared up-front; DMAs are ordered explicitly below so
            # the first nat loads lead the Pool queue.
            identity = consts.tile([128, 128], F32)
            wfin_sb = consts.tile([128, AJ], BF16)
            # W_enc f32->bf16 staging, full size: a single half-sized
            # buffer creates a WAR edge (half-2 DMA waits on half-1's casts)
            # that puts ~15us of W-load serialization on the startup path
            w_enc_sb = consts.tile([128, EJ * A], BF16)
            w8_sb = consts.tile([128, EJ * A], FP8)
            w_dec_sb = consts.tile([128, WJ * A], F32)
            ones32 = consts.tile([1, 32], F32)
            onescol = consts.tile([128, 1], BF16)
            dec_sb = consts.tile([B_LOC, W], F32)
            benc_sb = consts.tile([1, A], F32)
            bdec_sb = consts.tile([1, A], F32)
            bb_f = consts.tile([1, A], F32)
            decT_sb = consts.tile([128, WJ * B_LOC], F32)
            bias16_nat = consts.tile([B_LOC, A], BF16)
            ones32b = consts.tile([B_LOC, 512], BF16)
            attT_sb = consts.tile([128, NCHUNK], F32)
            out_sb = consts.tile([B_LOC, E], F32)
            recip_z = consts.tile([B_LOC, 1], F32)
            id4 = consts.tile([128, B_LOC], F32)
            masks_sb = consts.tile([128, NCHUNK * B_LOC], BF16)
            onesrow128 = consts.tile([1, 128], BF16)
            zrow = consts.tile([1, GW], BF16)

            # group structure: 2+3-chunk warmup groups, then GW-chunk groups
            sizes = [2, 3] + [GW] * ((NCHUNK - 5) // GW)
            if sum(sizes) < NCHUNK:
                sizes.append(NCHUNK - sum(sizes))
            assert sum(sizes) == NCHUNK
            starts = [sum(sizes[:i]) for i in range(len(sizes))]
            NG = len(sizes)

            with (
                tc.tile_pool(name="nat_pool", bufs=6) as nat_pool,
                tc.tile_pool(name="nat8_pool", bufs=6) as nat8_pool,
                tc.tile_pool(name="enc8_pool", bufs=4) as enc8_pool,
                tc.tile_pool(name="hidT_pool", bufs=2) as hidT_pool,
                tc.tile_pool(name="w6_pool", bufs=12) as w6_pool,
                tc.tile_pool(name="ind_pool", bufs=4) as ind_pool,
                tc.tile_pool(name="mm_ps", bufs=3, space="PSUM") as mm_ps,
                tc.tile_pool(name="acc_ps", bufs=1, space="PSUM") as acc_ps,
            ):
                nat = [None] * NCHUNK  # per-chunk [128, E] views into pair tiles
                enc8_of = [None] * NG
                ind_of = [None] * NG

                def issue_loads(g):
                    """nat cast-DMA (Pool) + transposes (sync) for group g,
                    plus the batch-membership indicator ind[b, r] = 1 iff
                    global row 128*cstart + r belongs to batch b (two Pool
                    affine_selects; iota = channel_multiplier*b + step*r +
                    base, TRUE keeps in_, upper bound via negated iota)."""
                    cstart, nch = starts[g], sizes[g]
                    gr = nch * 128
                    nat_t = nat_pool.tile([128, GW * E], BF16, name="nat")
                    for i in range(nch):
                        nat[cstart + i] = nat_t[:, i * E : (i + 1) * E]
                    # one cast-DMA per group (f32 HBM -> bf16 SBUF; splitting
                    # it finer costs more in per-DMA desc-gen + sem overhead
                    # than the earlier cast start saves). Desc-gen FIRST on
                    # the Pool queue: the ind selects can wait on an ind slot
                    # (freed by a PE bias matmul) and must not gate the load
                    # behind a PE stall
                    nc.gpsimd.dma_start(
                        nat_t.rearrange("p (i e) -> p i e", i=GW)[:, 0:nch, :],
                        enc_x[cstart * 128 : (cstart + nch) * 128, :].rearrange(
                            "(i p) e -> p i e", p=128, i=nch
                        ),
                    )
                    ind = ind_pool.tile([B_LOC, GR], BF16, name="ind")
                    ind_of[g] = ind
                    nc.gpsimd.affine_select(
                        ind[:, 0:gr], ones32b[0:B_LOC, 0:gr], pattern=[[1, gr]],
                        compare_op=mybir.AluOpType.is_ge, fill=0.0,
                        base=128 * cstart, channel_multiplier=-P,
                    )
                    nc.gpsimd.affine_select(
                        ind[:, 0:gr], ind[:, 0:gr], pattern=[[-1, gr]],
                        compare_op=mybir.AluOpType.is_ge, fill=0.0,
                        base=(P - 1) - 128 * cstart, channel_multiplier=P,
                    )

                # --- startup ordering ---------------------------------------
                # sync queue: small consts first so group-0 transposes follow
                nc.sync.dma_start(dec_sb[:], dec_x[:])
                nc.sync.dma_start(benc_sb[:], benc_x[:])
                nc.sync.dma_start(bdec_sb[:], bdec_x[:])
                # Pool queue: group-0 nat leads, then W halves; casts split
                # DVE/Act so the first matmul isn't gated on one engine
                nc.vector.memset(ones32b[:], 1.0)
                nc.vector.memset(onesrow128[:], 1.0)
                nc.vector.memset(zrow[:], 0.0)
                nc.gpsimd.dma_start(wfin_sb[:], wfin_x.rearrange("(j p) -> p j", p=128))
                HALF = EJ // 2 * A
                # (even,odd)-paired rows: e = 256k + 2p + t, matching the
                # uint16 pair-transpose layout of the fp8 activations
                nc.gpsimd.dma_start(
                    w_enc_sb[:, 0:HALF].rearrange("p (k t a) -> p k t a", t=2, a=A),
                    wenc_x[0 : EJ // 2 * 128, :].rearrange(
                        "(k p t) a -> p k t a", p=128, t=2
                    ),
                )
                nc.vector.tensor_scalar_mul(w8_sb[:, 0 : HALF // 2], w_enc_sb[:, 0 : HALF // 2], WSCALE)
                nc.scalar.activation(
                    w8_sb[:, HALF // 2 : HALF], w_enc_sb[:, HALF // 2 : HALF],
                    AF.Copy, scale=WSCALE,
                )
                nc.gpsimd.dma_start(
                    w_enc_sb[:, HALF : 2 * HALF].rearrange(
                        "p (k t a) -> p k t a", t=2, a=A
                    ),
                    wenc_x[EJ // 2 * 128 : EJ * 128, :].rearrange(
                        "(k p t) a -> p k t a", p=128, t=2
                    ),
                )
                nc.vector.tensor_scalar_mul(
                    w8_sb[:, HALF : HALF + HALF // 2],
                    w_enc_sb[:, HALF : HALF + HALF // 2], WSCALE,
                )
                nc.scalar.activation(
                    w8_sb[:, HALF + HALF // 2 : 2 * HALF],
                    w_enc_sb[:, HALF + HALF // 2 : 2 * HALF],
                    AF.Copy, scale=WSCALE,
                )
                # W_dec on the sync ring (f32; prologue matmul is tiny)
                nc.sync.dma_start(w_dec_sb[:], wdec_x.rearrange("(j p) a -> p j a", p=128))
                issue_loads(0)
                issue_loads(1)

                # small consts on compute engines
                make_identity(nc, identity[:])
                nc.vector.memset(ones32[:], 1.0)
                nc.vector.memset(onescol[:], 1.0)
                nc.gpsimd.memset(id4[:], 0.0)
                for k in range(4):
                    nc.gpsimd.affine_select(
                        id4[:], id4[:], pattern=[[-1, B_LOC]],
                        compare_op=mybir.AluOpType.not_equal, fill=1.0,
                        base=-B_LOC * k, channel_multiplier=1,
                    )
                nc.vector.tensor_add(bb_f[:], benc_sb[:], bdec_sb[:])

                for g in range(2, min(PREFETCH, NG)):
                    issue_loads(g)

                # all 49 chunk masks on Pool AFTER the prefetch desc-gens (10us
                # of selects must not block the early loads): mask_c[p, b] = 1
                # iff row 128c+p belongs to batch b (iota compare as in the
                # indicator build); the per-chunk w6 then costs one tiny Act
                # scaled copy — no Pool work on the steady-state critical path
                nc.gpsimd.memset(masks_sb[:], 1.0)
                for c in range(NCHUNK):
                    m = masks_sb[:, c * B_LOC : (c + 1) * B_LOC]
                    nc.gpsimd.affine_select(
                        m, m, pattern=[[-P, B_LOC]],
                        compare_op=mybir.AluOpType.is_ge, fill=0.0,
                        base=128 * c, channel_multiplier=1,
                    )
                    nc.gpsimd.affine_select(
                        m, m, pattern=[[P, B_LOC]],
                        compare_op=mybir.AluOpType.is_ge, fill=0.0,
                        base=(P - 1) - 128 * c, channel_multiplier=-1,
                    )

                # prologue: decT, then biasT = (dec @ W_dec + b_dec + b_enc).T
                # (all f32: the matmuls are tiny; PSUM rides the mm_ps ring)
                for j in range(WJ):
                    ps_dt = mm_ps.tile([128, B_LOC], F32, name="ps_h")
                    nc.tensor.transpose(
                        ps_dt[:], dec_sb[0:B_LOC, j * 128 : (j + 1) * 128],
                        identity[0:B_LOC, 0:B_LOC],
                    )
                    nc.vector.tensor_copy(decT_sb[:, j * B_LOC : (j + 1) * B_LOC], ps_dt[:])
                # bias rows in natural [b, a] layout, scaled by 16: the
                # per-batch bias is added on the PE as rank-1 matmuls appended
                # to each group's accumulation (so relus are segment-free one-
                # slice ops on either engine); the 1/16 moves into the exp
                ps_bn = mm_ps.tile([B_LOC, A], F32, name="ps_h")
                for wj in range(WJ):
                    nc.tensor.matmul(
                        ps_bn[:],
                        lhsT=decT_sb[:, wj * B_LOC : (wj + 1) * B_LOC],
                        rhs=w_dec_sb[:, wj * A : (wj + 1) * A],
                        start=(wj == 0), stop=False,
                    )
                nc.tensor.matmul(
                    ps_bn[:], lhsT=ones32[0:1, :], rhs=bb_f[0:1, :],
                    start=False, stop=True,
                )
                nc.scalar.activation(bias16_nat[:], ps_bn[:], AF.Copy, scale=WSCALE)

                # step6 uses PE column-packing: chunk c accumulates into
                # partition strip 32*(c%4) of full-height PSUM tensors; strips
                # are summed at the end.
                out_ps = [
                    acc_ps.tile([128, 512], F32, name=f"out_ps{eg}") for eg in range(EG)
                ]
                # z alone in its bank: interleaving the per-group at_ps
                # start/stop accumulations in the SAME PSUM bank as the
                # kernel-long z accumulation corrupts it on HW — at_ps rides
                # the mm_ps ring instead (at GW=4 its slot wait clears
                # mid-group, off the critical path)
                z_ps = acc_ps.tile([128, 1], F32)
                next6 = 0

                # static step6 release schedule: step4 runs one group late
                # (software-pipelined off the PE critical path), so chunk c's
                # attT lands during group h_c+1 and its weighted sum releases
                # at the top of group h_c+2; its w6 is built at the END of
                # group h_c+1, right after the exps on the same engine.
                group_of = {}
                for gi in range(NG):
                    for c in range(starts[gi], starts[gi] + sizes[gi]):
                        group_of[c] = gi
                released_at = [[] for _ in range(NG + 1)]
                for c in range(NCHUNK):
                    released_at[min(group_of[c] + 2, NG)].append(c)
                w6_of = [None] * NCHUNK

                def prepare_w6(chunks):
                    # w6 = mask_c * exp_att on the SAME engine as the exp
                    # (Act): it runs back-to-back after the exp with no
                    # cross-engine hop, and never queues behind the DVE's
                    # 3.2us nat8 casts
                    for c in chunks:
                        w6 = w6_pool.tile([128, B_LOC], BF16, name="w6")
                        w6_of[c] = w6
                        nc.scalar.activation(
                            w6[:], masks_sb[:, c * B_LOC : (c + 1) * B_LOC],
                            AF.Copy, scale=attT_sb[:, c : c + 1],
                        )

                def issue_step6(chunks):
                    nonlocal next6
                    for c in chunks:
                        w6 = w6_of[c]
                        # single strip: each chunk accumulates into rows
                        # 0..31 directly (RMW revisit distance is 5 matmuls,
                        # plenty); the cross-strip id4 reduction tail vanishes
                        for eg in range(EG):
                            nc.tensor.matmul(
                                out_ps[eg][0:B_LOC, :],
                                lhsT=w6[:],
                                rhs=nat[c][:, eg * 512 : (eg + 1) * 512],
                                start=(c == 0), stop=(c == NCHUNK - 1),
                            )
                        nc.tensor.matmul(
                            z_ps[0:B_LOC, :], lhsT=w6[:], rhs=onescol[:],
                            start=(c == 0), stop=(c == NCHUNK - 1),
                        )
                        next6 += 1

                w8_4d = w8_sb.rearrange("p (k t a) -> p k t a", k=EJ // 2, t=2)

                def issue_cast(g):
                    """fp8 cast in NATURAL layout (DVE/Act, off the DMA
                    pipe), then a half-width uint16 pair DMA-transpose:
                    enc8T block k holds e-pairs (256k+2p, 256k+2p+1) as
                    adjacent fp8 bytes, matching the paired W_enc layout.
                    Issued CAST_AHEAD groups before use."""
                    cstart, nch = starts[g], sizes[g]
                    enc8 = enc8_pool.tile([128, (EJ // 2) * 2 * GR], FP8, name="enc8")
                    enc8_of[g] = enc8
                    e8_u16_3d = enc8[:].bitcast(mybir.dt.uint16).rearrange(
                        "p (k r) -> p k r", k=EJ // 2
                    )
                    for i in range(nch):
                        c = cstart + i
                        nat8 = nat8_pool.tile([128, E], FP8, name="nat8")
                        # cast entirely on DVE: the Act queue carries the
                        # PE-dependent relu/exp/w6 chain, and an in-order Act
                        # would drag the prefetch casts behind it, lock-
                        # stepping the whole DMA pipeline to the PE
                        nc.vector.tensor_copy(nat8[:, 0:E], nat[c][:, 0:E])
                        # pair-transpose: u16 element = (e even, e odd) bytes.
                        # All transposes stay on ONE HWDGE ring: concurrent
                        # transposes on both rings corrupt data (shared xbar).
                        nc.sync.dma_start(
                            e8_u16_3d[:, :, i * 128 : i * 128 + 128],
                            nat8[:].bitcast(mybir.dt.uint16),
                            transpose=True,
                        )

                def issue_step4(cstart, nch, hidT):
                    """att columns for one group in ONE PSUM accumulation
                    group (single zero + single stop-sem), then per-chunk exp
                    (softmax numerator; no max-subtraction since |att| < ~2;
                    1/16 undoes the biasT scaling). Runs one group LATE so
                    the PE never stalls waiting for the relu chain."""
                    # rank-1 zeroing matmul writes ALL columns so every later
                    # accumulate lands on initialized PSUM (HW has no lazy
                    # zero-fill; a start=False first write reads garbage)
                    at_ps = mm_ps.tile([128, GW], F32, name="ps_h")
                    nc.tensor.matmul(
                        at_ps[:, 0:GW], lhsT=onesrow128[0:1, :],
                        rhs=zrow[0:1, 0:GW], start=True, stop=False,
                    )
                    for rc in range(nch):
                        for aj in range(AJ):
                            nc.tensor.matmul(
                                at_ps[:, rc : rc + 1],
                                lhsT=hidT[:, aj * GR + rc * 128 : aj * GR + rc * 128 + 128],
                                rhs=wfin_sb[:, aj : aj + 1],
                                start=False,
                                stop=(rc == nch - 1 and aj == AJ - 1),
                            )
                    for rc in range(nch):
                        c = cstart + rc
                        nc.scalar.activation(
                            attT_sb[:, c : c + 1], at_ps[:, rc : rc + 1],
                            AF.Exp, scale=1.0 / WSCALE,
                        )

                CAST_AHEAD = 2
                for gg in range(CAST_AHEAD):
                    issue_cast(gg)
                issued = min(PREFETCH, NG)
                pending4 = None  # (cstart, nch, hidT) of the previous group
                for g, (cstart, nch) in enumerate(zip(starts, sizes)):
                    gr = nch * 128
                    # weighted-sums for chunks completed two groups ago
                    # (w6 prepared at the end of group g-1): ready PE work
                    # while this group's data lands
                    issue_step6(released_at[g])

                    # previous group's att columns BEFORE this group's step1:
                    # hidT(g-1) is long ready, so these tiny matmuls fill the
                    # front of any enc8T stall and keep the PE clock warm; the
                    # at_ps ring slot they claim was freed a full group ago
                    if pending4 is not None:
                        issue_step4(*pending4)
                        pending4 = None

                    e8_4d = enc8_of[g].rearrange(
                        "p (k r t) -> p k t r", k=EJ // 2, t=2
                    )
                    hidT = hidT_pool.tile([128, AJ * GR], BF16, name="hidT")
                    for aj in range(AJ):
                        ps_h = mm_ps.tile([128, GR], F32, name="ps_h")
                        # per-batch bias via the indicator FIRST (ind/bias are
                        # ready long before enc8T, so the PE starts the
                        # accumulation without waiting on the transpose DMA)
                        nc.tensor.matmul(
                            ps_h[:, 0:gr],
                            lhsT=bias16_nat[0:B_LOC, aj * 128 : (aj + 1) * 128],
                            rhs=ind_of[g][:, 0:gr],
                            start=True, stop=False,
                        )
                        for t in range(EJ // 2):
                            nc.tensor.matmul(
                                ps_h[:, 0:gr],
                                lhsT=w8_4d[:, t, :, aj * 128 : (aj + 1) * 128],
                                rhs=e8_4d[:, t, :, 0:gr],
                                start=False, stop=(t == EJ // 2 - 1),
                                perf_mode=DR,
                            )
                        nc.scalar.activation(
                            hidT[:, aj * GR : aj * GR + gr],
                            ps_h[:, 0:gr], AF.Relu,
                        )

                    if g == NG - 1:
                        # last group: no next group to fill, issue its step4
                        # immediately so the tail is just w6 + step6 + out
                        issue_step4(cstart, nch, hidT)
                    else:
                        pending4 = (cstart, nch, hidT)

                    # build the NEXT group's w6 now (right after the exps on
                    # the same engine), then issue the next prefetch DMA
                    # behind it — the Pool-engine selects must never queue
                    # behind a 3us nat transfer
                    if g + 1 < NG:
                        prepare_w6(released_at[g + 1])
                    if issued < NG:
                        issue_loads(issued)
                        issued += 1

                    # lookahead cast last — 2 groups of slack keeps it off
                    # the critical path
                    if g + CAST_AHEAD < NG:
                        issue_cast(g + CAST_AHEAD)

                assert pending4 is None
                prepare_w6(released_at[NG])
                issue_step6(released_at[NG])
                assert next6 == NCHUNK
                # single-strip accumulators: scale by 1/Z, alternating
                # Act/DVE so the four copies pair up, each followed by its
                # own out-DMA slice so stores overlap the remaining copies
                nc.vector.reciprocal(recip_z[:], z_ps[0:B_LOC, :])
                for eg in range(EG):
                    dst = out_sb[:, eg * 512 : (eg + 1) * 512]
                    if eg % 2 == 0:
                        nc.scalar.activation(
                            dst, out_ps[eg][0:B_LOC, :], AF.Copy, scale=recip_z[:],
                        )
                    else:
                        nc.vector.tensor_scalar_mul(
                            dst, out_ps[eg][0:B_LOC, :], recip_z[:],
                        )
                    nc.sync.dma_start(
                        out_x[:, eg * 512 : (eg + 1) * 512], dst,
                    )

    nc.compile()
    return nc


_NC = None


def _get_nc():
    global _NC
    if _NC is None:
        _NC = build()
    return _NC


def _in_maps(inputs):
    enc = np.ascontiguousarray(np.asarray(inputs["encoder_out"], dtype=np.float32))
    dec = np.ascontiguousarray(np.asarray(inputs["decoder_out"], dtype=np.float32))
    wenc = np.ascontiguousarray(np.asarray(inputs["W_enc"], dtype=np.float32))
    benc = np.asarray(inputs["b_enc"], dtype=np.float32).reshape(1, A)
    wdec = np.ascontiguousarray(np.asarray(inputs["W_dec"], dtype=np.float32))
    bdec = np.asarray(inputs["b_dec"], dtype=np.float32).reshape(1, A)
    wfin = np.ascontiguousarray(np.asarray(inputs["W_fin"], dtype=np.float32))

    maps = []
    for i in range(N_CORES):
        maps.append(
            {
                "encoder_out": np.ascontiguousarray(
                    enc[i * B_LOC : (i + 1) * B_LOC].reshape(ROWS, E)
                ),
                "decoder_out": np.ascontiguousarray(dec[i * B_LOC : (i + 1) * B_LOC]),
                "W_enc": wenc,
                "b_enc": benc,
                "W_dec": wdec,
                "b_dec": bdec,
                "W_fin": wfin,
            }
        )
    return maps


def run(inputs, trace=False):
    """Run the kernel; returns (out [256, 2048] f32, exec_time_ns or None)."""
    nc = _get_nc()
    res = run_bass_kernel_spmd(
        nc, _in_maps(inputs), core_ids=list(range(N_CORES)), trace=trace
    )
    out = np.concatenate([res.results[i]["out"] for i in range(N_CORES)], axis=0)
    return out.astype(np.float32), res.exec_time_ns


def kernel(**inputs):
    out, _ = run(inputs, trace=False)
    return out



# revision 55
# speedup vs baseline: 1.3507x; 1.3507x over previous
"""Trainium2 Bass kernel for the additive-attention module.

Per-core computation (data-parallel over batch, 8 cores, 32 batches each):
  att_enc = enc @ W_enc            [6272, 2048] @ [2048, 512]  (dominant matmul)
  att_dec = dec @ W_dec + b_dec    [32, 512] @ [512, 512]
  hidden  = relu(att_enc + att_dec[b] + b_enc)
  att     = hidden @ W_fin         (b_fin dropped: softmax shift-invariant)
  w       = softmax_p(att)         (no max-subtraction: |att| < ~2)
  out     = sum_p w[b,p] * enc[b,p,:]

Dataflow per 128-row chunk c (4-chunk groups; nat DMA 3 groups ahead, fp8
cast + transpose 2 groups ahead; 26 groups total vs 26+ at GW=2 — bigger
groups amortize the per-group DMA->cast->transpose latency chain, which the
tile scheduler otherwise lock-steps against the PE):
  nat_c   [128 rows, 2048 e] bf16   gpsimd cast-DMA from f32 HBM (one DMA
                                    per group: finer splits cost more in
                                    desc-gen + sem overhead than they save)
  nat8_c  [128 rows, 2048 e] fp8e4  DVE-only casts of nat (the Act queue
                                    carries the PE-dependent relu/exp/w6
                                    chain; in-order Act would drag these
                                    prefetch casts behind it and lock-step
                                    the DMA pipeline to the PE)
  enc8T   [128 e-pairs, ...] fp8e4  half-width uint16 pair DMA-transpose of
                                    nat8: block k holds e-pairs (256k+2p,
                                    256k+2p+1) as adjacent bytes, matching
                                    the (even,odd)-row-paired W_enc load
  step1: att_encT accumulated per aj over bias-indicator matmul FIRST
         (bias16.T @ ind is ready long before enc8T, so the PE opens the
         accumulation without waiting on the transpose) + 8 fp8 DoubleRow
         k-pair matmuls (W_enc pre-scaled x16 into fp8, staged through a
         full-size bf16 buffer; segment-free relus on Act) -> hidT bf16
  step4: att columns for group g-1, issued at the TOP of group g (hidT(g-1)
         is long ready, so these tiny matmuls fill the front of any enc8T
         stall and keep the PE p-state warm): ONE PSUM accumulation group
         (rank-1 zeroing matmul covers all columns first — a start=False
         first write would read uninitialized PSUM on HW; at_ps rides the
         mm_ps ring — packing it into the z bank corrupts the kernel-long z
         accumulation on HW); exp with scale=1/16 -> attT f32
  step6: w6 = mask_c * exp column, built on the SAME engine as the exp
         (Act, zero cross-engine hop) from prologue-precomputed masks
         (built AFTER the prefetch desc-gens — 10us of Pool selects must
         not block the early loads); chunk c releases at group h_c + 2; all
         chunks accumulate single-strip into out_ps[eg][0:32] / z_ps, and
         the tail is just reciprocal + alternating Act/DVE scaled copies,
         each overlapped with its own out-DMA slice.
"""

import sys

try:
    import concourse.bass as bass  # noqa: F401
except ImportError:
    sys.path.insert(0, "/opt/trn_rl_repo")

import numpy as np

import concourse.bass as bass
import concourse.mybir as mybir
import concourse.tile as tile
from concourse import bacc
from concourse.bass_utils import run_bass_kernel_spmd
from concourse.masks import make_identity

F32 = mybir.dt.float32
BF16 = mybir.dt.bfloat16
FP8 = mybir.dt.float8e4
AF = mybir.ActivationFunctionType
DR = mybir.MatmulPerfMode.DoubleRow

N_CORES = 8
B = 256
B_LOC = B // N_CORES  # 32
P = 196
E = 2048
A = 512
W = 512
ROWS = B_LOC * P  # 6272
NCHUNK = (ROWS + 127) // 128  # 49
EJ = E // 128  # 16
AJ = A // 128  # 4
WJ = W // 128  # 4
EG = E // 512  # 4
WSCALE = 16.0  # fp8 W_enc pre-scale (undone in the relu activation)
GW = 6  # chunks per steady-state group
PREFETCH = 2  # groups of nat DMA issued ahead of compute (nat ring holds 5:
# window = PREFETCH + 2 groups of liveness (step6 runs at h+2) + 1 slack)


def _batch_segments(r0, nrows):
    """Batch segments of global row range [r0, r0+nrows): (batch, local_s0, local_s1)."""
    segs = []
    b0 = r0 // P
    b1 = (r0 + nrows - 1) // P
    for b in range(b0, b1 + 1):
        s0 = max(b * P - r0, 0)
        s1 = min((b + 1) * P - r0, nrows)
        if s1 > s0:
            segs.append((b, s0, s1))
    return segs


def build():
    nc = bacc.Bacc()

    enc_x = nc.dram_tensor("encoder_out", [ROWS, E], F32, kind="ExternalInput")
    dec_x = nc.dram_tensor("decoder_out", [B_LOC, W], F32, kind="ExternalInput")
    wenc_x = nc.dram_tensor("W_enc", [E, A], F32, kind="ExternalInput")
    benc_x = nc.dram_tensor("b_enc", [1, A], F32, kind="ExternalInput")
    wdec_x = nc.dram_tensor("W_dec", [W, A], F32, kind="ExternalInput")
    bdec_x = nc.dram_tensor("b_dec", [1, A], F32, kind="ExternalInput")
    wfin_x = nc.dram_tensor("W_fin", [A], F32, kind="ExternalInput")
    out_x = nc.dram_tensor("out", [B_LOC, E], F32, kind="ExternalOutput")

    GR = GW * 128  # steady-state group row width

    with tile.TileContext(nc) as tc:
        with tc.tile_pool(name="consts", bufs=1) as consts:
            # tiles declared up-front; DMAs are ordered explicitly below so
            # the first nat loads lead the Pool queue.
            identity = consts.tile([128, 128], F32)
            wfin_sb = consts.tile([128, AJ], BF16)
            w8_sb = consts.tile([128, EJ * A], FP8)
            w_dec_sb = consts.tile([128, WJ * A], F32)
            ones32 = consts.tile([1, 32], F32)
            onescol = consts.tile([128, 1], BF16)
            dec_sb = consts.tile([B_LOC, W], F32)
            benc_sb = consts.tile([1, A], F32)
            bdec_sb = consts.tile([1, A], F32)
            bb_f = consts.tile([1, A], F32)
            decT_sb = consts.tile([128, WJ * B_LOC], F32)
            bias16_nat = consts.tile([B_LOC, A], BF16)
            ones32b = consts.tile([B_LOC, GR], BF16)
            attT_sb = consts.tile([128, NCHUNK], F32)
            out_sb = consts.tile([B_LOC, E], F32)
            recip_z = consts.tile([B_LOC, 1], F32)
            id4 = consts.tile([128, B_LOC], F32)
            masks_sb = consts.tile([128, NCHUNK * B_LOC], BF16)
            onesrow128 = consts.tile([1, 128], BF16)
            zrow = consts.tile([1, GW], BF16)

            # group structure: 2+3-chunk warmup groups, then GW-chunk groups
            sizes = [2, 3] + [GW] * ((NCHUNK - 5) // GW)
            if sum(sizes) < NCHUNK:
                sizes.append(NCHUNK - sum(sizes))
            assert sum(sizes) == NCHUNK
            starts = [sum(sizes[:i]) for i in range(len(sizes))]
            NG = len(sizes)

            with (
                tc.tile_pool(name="nat_pool", bufs=4) as nat_pool,
                tc.tile_pool(name="nat8_pool", bufs=4) as nat8_pool,
                tc.tile_pool(name="enc8_pool", bufs=3) as enc8_pool,
                tc.tile_pool(name="hidT_pool", bufs=2) as hidT_pool,
                tc.tile_pool(name="w6_pool", bufs=12) as w6_pool,
                tc.tile_pool(name="ind_pool", bufs=4) as ind_pool,
                tc.tile_pool(name="mm_ps", bufs=3, space="PSUM") as mm_ps,
                tc.tile_pool(name="acc_ps", bufs=1, space="PSUM") as acc_ps,
            ):
                nat = [None] * NCHUNK  # per-chunk [128, E] views into pair tiles
                enc8_of = [None] * NG
                ind_of = [None] * NG

                def issue_loads(g):
                    """nat cast-DMA (Pool) + transposes (sync) for group g,
                    plus the batch-membership indicator ind[b, r] = 1 iff
                    global row 128*cstart + r belongs to batch b (two Pool
                    affine_selects; iota = channel_multiplier*b + step*r +
                    base, TRUE keeps in_, upper bound via negated iota)."""
                    cstart, nch = starts[g], sizes[g]
                    gr = nch * 128
                    nat_t = nat_pool.tile([128, GW * E], BF16, name="nat")
                    for i in range(nch):
                        nat[cstart + i] = nat_t[:, i * E : (i + 1) * E]
                    # one cast-DMA per group (f32 HBM -> bf16 SBUF; splitting
                    # it finer costs more in per-DMA desc-gen + sem overhead
                    # than the earlier cast start saves). Desc-gen FIRST on
                    # the Pool queue: the ind selects can wait on an ind slot
                    # (freed by a PE bias matmul) and must not gate the load
                    # behind a PE stall
                    nc.gpsimd.dma_start(
                        nat_t.rearrange("p (i e) -> p i e", i=GW)[:, 0:nch, :],
                        enc_x[cstart * 128 : (cstart + nch) * 128, :].rearrange(
                            "(i p) e -> p i e", p=128, i=nch
                        ),
                    )
                    ind = ind_pool.tile([B_LOC, GR], BF16, name="ind")
                    ind_of[g] = ind
                    nc.gpsimd.affine_select(
                        ind[:, 0:gr], ones32b[0:B_LOC, 0:gr], pattern=[[1, gr]],
                        compare_op=mybir.AluOpType.is_ge, fill=0.0,
                        base=128 * cstart, channel_multiplier=-P,
                    )
                    nc.gpsimd.affine_select(
                        ind[:, 0:gr], ind[:, 0:gr], pattern=[[-1, gr]],
                        compare_op=mybir.AluOpType.is_ge, fill=0.0,
                        base=(P - 1) - 128 * cstart, channel_multiplier=P,
                    )

                # --- startup ordering ---------------------------------------
                # sync queue: small consts first so group-0 transposes follow
                nc.sync.dma_start(dec_sb[:], dec_x[:])
                nc.sync.dma_start(benc_sb[:], benc_x[:])
                nc.sync.dma_start(bdec_sb[:], bdec_x[:])
                # Pool queue: group-0 nat leads, then W halves; casts split
                # DVE/Act so the first matmul isn't gated on one engine
                nc.vector.memset(ones32b[:], 1.0)
                nc.vector.memset(onesrow128[:], 1.0)
                nc.vector.memset(zrow[:], 0.0)
                # W_enc f32->bf16 staging borrows a nat-ring slot (24KB >=
                # the 16KB needed): it is dead once the w8 casts finish, and
                # the ring's WAR edge recycles the slot for a later group —
                # 16KB/partition of SBUF that a permanent const would pin
                w_enc_sb = nat_pool.tile([128, GW * E], BF16, name="nat")
                nc.gpsimd.dma_start(wfin_sb[:], wfin_x.rearrange("(j p) -> p j", p=128))
                HALF = EJ // 2 * A
                # (even,odd)-paired rows: e = 256k + 2p + t, matching the
                # uint16 pair-transpose layout of the fp8 activations
                nc.gpsimd.dma_start(
                    w_enc_sb[:, 0:HALF].rearrange("p (k t a) -> p k t a", t=2, a=A),
                    wenc_x[0 : EJ // 2 * 128, :].rearrange(
                        "(k p t) a -> p k t a", p=128, t=2
                    ),
                )
                nc.vector.tensor_scalar_mul(w8_sb[:, 0 : HALF // 2], w_enc_sb[:, 0 : HALF // 2], WSCALE)
                nc.scalar.activation(
                    w8_sb[:, HALF // 2 : HALF], w_enc_sb[:, HALF // 2 : HALF],
                    AF.Copy, scale=WSCALE,
                )
                nc.gpsimd.dma_start(
                    w_enc_sb[:, HALF : 2 * HALF].rearrange(
                        "p (k t a) -> p k t a", t=2, a=A
                    ),
                    wenc_x[EJ // 2 * 128 : EJ * 128, :].rearrange(
                        "(k p t) a -> p k t a", p=128, t=2
                    ),
                )
                nc.vector.tensor_scalar_mul(
                    w8_sb[:, HALF : HALF + HALF // 2],
                    w_enc_sb[:, HALF : HALF + HALF // 2], WSCALE,
                )
                nc.scalar.activation(
                    w8_sb[:, HALF + HALF // 2 : 2 * HALF],
                    w_enc_sb[:, HALF + HALF // 2 : 2 * HALF],
                    AF.Copy, scale=WSCALE,
                )
                # W_dec on the sync ring (f32; prologue matmul is tiny)
                nc.sync.dma_start(w_dec_sb[:], wdec_x.rearrange("(j p) a -> p j a", p=128))
                issue_loads(0)
                issue_loads(1)

                # small consts on compute engines
                make_identity(nc, identity[:])
                nc.vector.memset(ones32[:], 1.0)
                nc.vector.memset(onescol[:], 1.0)
                nc.gpsimd.memset(id4[:], 0.0)
                for k in range(4):
                    nc.gpsimd.affine_select(
                        id4[:], id4[:], pattern=[[-1, B_LOC]],
                        compare_op=mybir.AluOpType.not_equal, fill=1.0,
                        base=-B_LOC * k, channel_multiplier=1,
                    )
                nc.vector.tensor_add(bb_f[:], benc_sb[:], bdec_sb[:])

                for g in range(2, min(PREFETCH, NG)):
                    issue_loads(g)

                # all 49 chunk masks on Pool AFTER the prefetch desc-gens (10us
                # of selects must not block the early loads): mask_c[p, b] = 1
                # iff row 128c+p belongs to batch b (iota compare as in the
                # indicator build); the per-chunk w6 then costs one tiny Act
                # scaled copy — no Pool work on the steady-state critical path
                nc.gpsimd.memset(masks_sb[:], 1.0)
                for c in range(NCHUNK):
                    m = masks_sb[:, c * B_LOC : (c + 1) * B_LOC]
                    nc.gpsimd.affine_select(
                        m, m, pattern=[[-P, B_LOC]],
                        compare_op=mybir.AluOpType.is_ge, fill=0.0,
                        base=128 * c, channel_multiplier=1,
                    )
                    nc.gpsimd.affine_select(
                        m, m, pattern=[[P, B_LOC]],
                        compare_op=mybir.AluOpType.is_ge, fill=0.0,
                        base=(P - 1) - 128 * c, channel_multiplier=-1,
                    )

                # prologue: decT, then biasT = (dec @ W_dec + b_dec + b_enc).T
                # (all f32: the matmuls are tiny; PSUM rides the mm_ps ring)
                for j in range(WJ):
                    ps_dt = mm_ps.tile([128, B_LOC], F32, name="ps_h")
                    nc.tensor.transpose(
                        ps_dt[:], dec_sb[0:B_LOC, j * 128 : (j + 1) * 128],
                        identity[0:B_LOC, 0:B_LOC],
                    )
                    nc.vector.tensor_copy(decT_sb[:, j * B_LOC : (j + 1) * B_LOC], ps_dt[:])
                # bias rows in natural [b, a] layout, scaled by 16: the
                # per-batch bias is added on the PE as rank-1 matmuls appended
                # to each group's accumulation (so relus are segment-free one-
                # slice ops on either engine); the 1/16 moves into the exp
                ps_bn = mm_ps.tile([B_LOC, A], F32, name="ps_h")
                for wj in range(WJ):
                    nc.tensor.matmul(
                        ps_bn[:],
                        lhsT=decT_sb[:, wj * B_LOC : (wj + 1) * B_LOC],
                        rhs=w_dec_sb[:, wj * A : (wj + 1) * A],
                        start=(wj == 0), stop=False,
                    )
                nc.tensor.matmul(
                    ps_bn[:], lhsT=ones32[0:1, :], rhs=bb_f[0:1, :],
                    start=False, stop=True,
                )
                nc.scalar.activation(bias16_nat[:], ps_bn[:], AF.Copy, scale=WSCALE)

                # step6 uses PE column-packing: chunk c accumulates into
                # partition strip 32*(c%4) of full-height PSUM tensors; strips
                # are summed at the end.
                out_ps = [
                    acc_ps.tile([128, 512], F32, name=f"out_ps{eg}") for eg in range(EG)
                ]
                # z alone in its bank: interleaving the per-group at_ps
                # start/stop accumulations in the SAME PSUM bank as the
                # kernel-long z accumulation corrupts it on HW — at_ps rides
                # the mm_ps ring instead (at GW=4 its slot wait clears
                # mid-group, off the critical path)
                z_ps = acc_ps.tile([128, 1], F32)
                next6 = 0

                # static step6 release schedule: step4 runs one group late
                # (software-pipelined off the PE critical path), so chunk c's
                # attT lands during group h_c+1 and its weighted sum releases
                # at the top of group h_c+2; its w6 is built at the END of
                # group h_c+1, right after the exps on the same engine.
                group_of = {}
                for gi in range(NG):
                    for c in range(starts[gi], starts[gi] + sizes[gi]):
                        group_of[c] = gi
                released_at = [[] for _ in range(NG + 1)]
                for c in range(NCHUNK):
                    # h+1: step4(h) runs at the top of group h+1, so chunk
                    # c's w6 and weighted sum follow in the SAME group —
                    # one group less of nat liveness and a shorter tail
                    released_at[min(group_of[c] + 1, NG)].append(c)
                w6_of = [None] * NCHUNK

                def prepare_w6(chunks):
                    # w6 = mask_c * exp_att on the SAME engine as the exp
                    # (Act): it runs back-to-back after the exp with no
                    # cross-engine hop, and never queues behind the DVE's
                    # 3.2us nat8 casts
                    for c in chunks:
                        w6 = w6_pool.tile([128, B_LOC], BF16, name="w6")
                        w6_of[c] = w6
                        nc.scalar.activation(
                            w6[:], masks_sb[:, c * B_LOC : (c + 1) * B_LOC],
                            AF.Copy, scale=attT_sb[:, c : c + 1],
                        )

                def issue_step6(chunks):
                    nonlocal next6
                    for c in chunks:
                        w6 = w6_of[c]
                        # single strip: each chunk accumulates into rows
                        # 0..31 directly (RMW revisit distance is 5 matmuls,
                        # plenty); the cross-strip id4 reduction tail vanishes
                        for eg in range(EG):
                            nc.tensor.matmul(
                                out_ps[eg][0:B_LOC, :],
                                lhsT=w6[:],
                                rhs=nat[c][:, eg * 512 : (eg + 1) * 512],
                                start=(c == 0), stop=(c == NCHUNK - 1),
                            )
                        nc.tensor.matmul(
                            z_ps[0:B_LOC, :], lhsT=w6[:], rhs=onescol[:],
                            start=(c == 0), stop=(c == NCHUNK - 1),
                        )
                        next6 += 1

                w8_4d = w8_sb.rearrange("p (k t a) -> p k t a", k=EJ // 2, t=2)

                def issue_cast(g):
                    """fp8 cast in NATURAL layout (DVE/Act, off the DMA
                    pipe), then a half-width uint16 pair DMA-transpose:
                    enc8T block k holds e-pairs (256k+2p, 256k+2p+1) as
                    adjacent fp8 bytes, matching the paired W_enc layout.
                    Issued CAST_AHEAD groups before use."""
                    cstart, nch = starts[g], sizes[g]
                    enc8 = enc8_pool.tile([128, (EJ // 2) * 2 * GR], FP8, name="enc8")
                    enc8_of[g] = enc8
                    e8_u16_3d = enc8[:].bitcast(mybir.dt.uint16).rearrange(
                        "p (k r) -> p k r", k=EJ // 2
                    )
                    for i in range(nch):
                        c = cstart + i
                        nat8 = nat8_pool.tile([128, E], FP8, name="nat8")
                        # cast entirely on DVE: the Act queue carries the
                        # PE-dependent relu/exp/w6 chain, and an in-order Act
                        # would drag the prefetch casts behind it, lock-
                        # stepping the whole DMA pipeline to the PE
                        nc.vector.tensor_copy(nat8[:, 0:E], nat[c][:, 0:E])
                        # pair-transpose: u16 element = (e even, e odd) bytes.
                        # All transposes stay on ONE HWDGE ring: concurrent
                        # transposes on both rings corrupt data (shared xbar).
                        nc.sync.dma_start(
                            e8_u16_3d[:, :, i * 128 : i * 128 + 128],
                            nat8[:].bitcast(mybir.dt.uint16),
                            transpose=True,
                        )

                def issue_step4(cstart, nch, hidT):
                    """att columns for one group in ONE PSUM accumulation
                    group (single zero + single stop-sem), then per-chunk exp
                    (softmax numerator; no max-subtraction since |att| < ~2;
                    1/16 undoes the biasT scaling). Runs one group LATE so
                    the PE never stalls waiting for the relu chain."""
                    # rank-1 zeroing matmul writes ALL columns so every later
                    # accumulate lands on initialized PSUM (HW has no lazy
                    # zero-fill; a start=False first write reads garbage)
                    at_ps = mm_ps.tile([128, GW], F32, name="ps_h")
                    nc.tensor.matmul(
                        at_ps[:, 0:GW], lhsT=onesrow128[0:1, :],
                        rhs=zrow[0:1, 0:GW], start=True, stop=False,
                    )
                    for rc in range(nch):
                        for aj in range(AJ):
                            nc.tensor.matmul(
                                at_ps[:, rc : rc + 1],
                                lhsT=hidT[:, aj * GR + rc * 128 : aj * GR + rc * 128 + 128],
                                rhs=wfin_sb[:, aj : aj + 1],
                                start=False,
                                stop=(rc == nch - 1 and aj == AJ - 1),
                            )
                    for rc in range(nch):
                        c = cstart + rc
                        nc.scalar.activation(
                            attT_sb[:, c : c + 1], at_ps[:, rc : rc + 1],
                            AF.Exp, scale=1.0 / WSCALE,
                        )

                CAST_AHEAD = 2
                for gg in range(CAST_AHEAD):
                    issue_cast(gg)
                issued = min(PREFETCH, NG)
                pending4 = None  # (cstart, nch, hidT) of the previous group
                for g, (cstart, nch) in enumerate(zip(starts, sizes)):
                    gr = nch * 128
                    # prefetch DMA first: its desc-gen priority precedes the
                    # whole group body so the scheduler can run it early
                    if issued < NG:
                        issue_loads(issued)
                        issued += 1
                    # previous group's att columns FIRST (hidT(g-1) is long
                    # ready, so these tiny matmuls fill the front of any
                    # enc8T stall and keep the PE clock warm), then its w6
                    # right behind the exps on Act, then its weighted sums —
                    # all BEFORE this group's step1 in the PE queue
                    if pending4 is not None:
                        issue_step4(*pending4)
                        pending4 = None
                    prepare_w6(released_at[g])

                    e8_4d = enc8_of[g].rearrange(
                        "p (k r t) -> p k t r", k=EJ // 2, t=2
                    )
                    hidT = hidT_pool.tile([128, AJ * GR], BF16, name="hidT")
                    HGR = GR // 2  # PSUM sub-block: GR f32 would span banks
                    for aj in range(AJ):
                        for sbk in range(2):
                            lo, hi = sbk * HGR, min(gr, (sbk + 1) * HGR)
                            if hi <= lo:
                                continue
                            ps_h = mm_ps.tile([128, HGR], F32, name="ps_h")
                            # per-batch bias via the indicator FIRST (ind/
                            # bias are ready long before enc8T, so the PE
                            # starts the accumulation without waiting on the
                            # transpose DMA)
                            nc.tensor.matmul(
                                ps_h[:, 0 : hi - lo],
                                lhsT=bias16_nat[0:B_LOC, aj * 128 : (aj + 1) * 128],
                                rhs=ind_of[g][:, lo:hi],
                                start=True, stop=False,
                            )
                            for t in range(EJ // 2):
                                nc.tensor.matmul(
                                    ps_h[:, 0 : hi - lo],
                                    lhsT=w8_4d[:, t, :, aj * 128 : (aj + 1) * 128],
                                    rhs=e8_4d[:, t, :, lo:hi],
                                    start=False, stop=(t == EJ // 2 - 1),
                                    perf_mode=DR,
                                )
                            nc.scalar.activation(
                                hidT[:, aj * GR + lo : aj * GR + hi],
                                ps_h[:, 0 : hi - lo], AF.Relu,
                            )

                    if g == NG - 1:
                        # last group: no next group to fill, issue its step4
                        # immediately so the tail is just w6 + step6 + out
                        issue_step4(cstart, nch, hidT)
                    else:
                        pending4 = (cstart, nch, hidT)
                    # weighted sums AFTER step1: their w6 (exps ran during
                    # step1) is ready by now, so no PE stall at the group top
                    issue_step6(released_at[g])

                    # build the NEXT group's w6 now (right after the exps on
                    # the same engine), then issue the next prefetch DMA
                    # behind it — the Pool-engine selects must never queue
                    # behind a 3us nat transfer

                    # lookahead cast last — 2 groups of slack keeps it off
                    # the critical path
                    if g + CAST_AHEAD < NG:
                        issue_cast(g + CAST_AHEAD)

                assert pending4 is None
                prepare_w6(released_at[NG])
                issue_step6(released_at[NG])
                assert next6 == NCHUNK
                # single-strip accumulators: scale by 1/Z, alternating
                # Act/DVE so the four copies pair up, each followed by its
                # own out-DMA slice so stores overlap the remaining copies
                nc.vector.reciprocal(recip_z[:], z_ps[0:B_LOC, :])
                for eg in range(EG):
                    dst = out_sb[:, eg * 512 : (eg + 1) * 512]
                    if eg % 2 == 0:
                        nc.scalar.activation(
                            dst, out_ps[eg][0:B_LOC, :], AF.Copy, scale=recip_z[:],
                        )
                    else:
                        nc.vector.tensor_scalar_mul(
                            dst, out_ps[eg][0:B_LOC, :], recip_z[:],
                        )
                    nc.sync.dma_start(
                        out_x[:, eg * 512 : (eg + 1) * 512], dst,
                    )

    nc.compile()
    return nc


_NC = None


def _get_nc():
    global _NC
    if _NC is None:
        _NC = build()
    return _NC


def _in_maps(inputs):
    enc = np.ascontiguousarray(np.asarray(inputs["encoder_out"], dtype=np.float32))
    dec = np.ascontiguousarray(np.asarray(inputs["decoder_out"], dtype=np.float32))
    wenc = np.ascontiguousarray(np.asarray(inputs["W_enc"], dtype=np.float32))
    benc = np.asarray(inputs["b_enc"], dtype=np.float32).reshape(1, A)
    wdec = np.ascontiguousarray(np.asarray(inputs["W_dec"], dtype=np.float32))
    bdec = np.asarray(inputs["b_dec"], dtype=np.float32).reshape(1, A)
    wfin = np.ascontiguousarray(np.asarray(inputs["W_fin"], dtype=np.float32))

    maps = []
    for i in range(N_CORES):
        maps.append(
            {
                "encoder_out": np.ascontiguousarray(
                    enc[i * B_LOC : (i + 1) * B_LOC].reshape(ROWS, E)
                ),
                "decoder_out": np.ascontiguousarray(dec[i * B_LOC : (i + 1) * B_LOC]),
                "W_enc": wenc,
                "b_enc": benc,
                "W_dec": wdec,
                "b_dec": bdec,
                "W_fin": wfin,
            }
        )
    return maps


def run(inputs, trace=False):
    """Run the kernel; returns (out [256, 2048] f32, exec_time_ns or None)."""
    nc = _get_nc()
    res = run_bass_kernel_spmd(
        nc, _in_maps(inputs), core_ids=list(range(N_CORES)), trace=trace
    )
    out = np.concatenate([res.results[i]["out"] for i in range(N_CORES)], axis=0)
    return out.astype(np.float32), res.exec_time_ns


def kernel(**inputs):
    out, _ = run(inputs, trace=False)
    return out

